# revision 67
# baseline (speedup 1.0000x reference)
"""Trainium2 Bass kernel for nn_CADense (context-adaptive low-rank dense layer).

Computes, for the full batch:
    s_mod = s + context @ w          # [B, R]
    low   = (data @ u) * s_mod       # [B, R]
    out   = relu(low @ v.T + 2*bias) # [B, UNITS]

Sharding: data-parallel over batch across 8 NeuronCores; u/s/v/w/bias
replicated. Each core runs the same Bass program on its 1024-row shard.

The kernel is HBM-traffic bound at fp32 (23.6 MB/core ≈ 60+ us at per-core
HBM bandwidth), so all matmul operands are marshaled to bf16 on the host
and the output is stored bf16 (widened to fp32 host-side): 11.8 MB/core.
All accumulation stays fp32 in PSUM; measured end-to-end rel err ~1e-3
vs the 2e-2 gate. bias is all-zeros per the spec; a nonzero bias falls
back to an exact host computation.

Host-side marshaling lays every tensor out exactly as its SBUF tile
([128, ...] partition-major, contraction-dim-major free layout), so each
DMA is one contiguous segment per partition — fat descriptors, cheap
HWDGE dispatch. Loads are split across both HWDGE rings, balanced
byte-for-byte in consumption order; stores ride the gpsimd SWDGE ring
(its expensive descriptor drain then overlaps the PE tail), with the
last stores on the by-then-idle HWDGE rings.

Compute per 512-row batch tile, in the transposed domain per rank-chunk:
    pd[r, b]  = (u.T @ data.T)[r, b]          (16 K-chunks into PSUM)
    ps[r, b]  = (w.T @ ctx.T)[r, b]           (4 K-chunks into PSUM)
    smod[r,b] = ps[r,b] + s[r]                (scalar-engine evacuation)
    lowT[r,b] = pd[r,b] * smod[r,b]           (DVE, bf16 out)
    out[b, m] = relu(lowT.T @ v.T)            (PSUM; scalar/vector ReLU evac)
PSUM banks: 2 for pd, 1 for ps (the two rank halves of smod serialize
through it), 5 for the output-stage ring (more slots = less matmul
stalling on ReLU evacuation). The batch-tile-1 rank stage is interleaved
with batch-tile-0's output stage so the PE rarely idles long enough for
the HAM clock gate to re-throttle; a short burst of dummy bf16 matmuls
pre-warms the gate while the first DMAs stream in.
"""

import os
import sys
from contextlib import ExitStack

import numpy as np

try:
    import ml_dtypes  # noqa: F401

    BF16_NP = np.dtype("bfloat16")
except (ImportError, TypeError):
    from jax import numpy as _jnp  # pragma: no cover

    BF16_NP = _jnp.bfloat16


def _ensure_concourse():
    try:
        import concourse  # noqa: F401
    except ImportError:
        for p in ("/opt/trn_rl_repo", "/root/.axon_site/_ro/trn_rl_repo"):
            if os.path.isdir(p) and p not in sys.path:
                sys.path.insert(0, p)


_ensure_concourse()

import concourse.tile as tile  # noqa: E402
from concourse import bacc, mybir  # noqa: E402
from concourse.bass_utils import run_bass_kernel_spmd  # noqa: E402

NCORES = 8
B, N_IN, UNITS, RANK, CCTX = 8192, 2048, 2048, 256, 512
NB = B // NCORES  # batch rows per core
P = 128
BT = 512  # batch tile (free dim of T-domain matmuls)
NBT = NB // BT  # batch tiles per core
KC = N_IN // P  # 16 contraction chunks for data @ u
CC = CCTX // P  # 4 contraction chunks for context @ w
RC = RANK // P  # 2 rank chunks
MS = 512  # output units slice width
NMS = UNITS // MS  # 4 unit slices
# data DMA chunking (in KC units) per batch tile: small head chunks so the
# first rank matmuls start early, bigger ones once the pipe is primed.
GROUPS0 = (2, 2, 4, 4, 4)
GROUPS1 = (8, 8)
N_WARMUP_MM = 34
WU_N = 256  # warmup matmul free dim

F32 = mybir.dt.float32
BF16 = mybir.dt.bfloat16
RELU = mybir.ActivationFunctionType.Relu
ADD = mybir.AluOpType.add
MULT = mybir.AluOpType.mult


def _emit(nc, tc, ctx):
    d_dataT = nc.dram_tensor("dataT", [P, NBT, KC, BT], BF16, kind="ExternalInput")
    d_ctxT = nc.dram_tensor("ctxT", [P, NBT, CC, BT], BF16, kind="ExternalInput")
    d_u = nc.dram_tensor("u", [P, KC, RANK], BF16, kind="ExternalInput")
    d_s = nc.dram_tensor("s", [P, RC], F32, kind="ExternalInput")
    d_vT = nc.dram_tensor("vT", [P, 2, RC, UNITS // 2], BF16, kind="ExternalInput")
    d_w = nc.dram_tensor("w", [P, CC, RANK], BF16, kind="ExternalInput")
    d_out = nc.dram_tensor("out", [NB, UNITS], BF16, kind="ExternalOutput")

    singles = ctx.enter_context(tc.tile_pool(name="singles", bufs=1))
    du_psum = ctx.enter_context(tc.tile_pool(name="du_psum", bufs=2, space="PSUM"))
    s_psum = ctx.enter_context(tc.tile_pool(name="s_psum", bufs=1, space="PSUM"))
    o_psum = ctx.enter_context(tc.tile_pool(name="o_psum", bufs=5, space="PSUM"))

    # ---- SBUF tiles (all single-use: no pool-recycle stalls on DMA rings) --
    u_sb = singles.tile([P, KC, RANK], BF16)
    w_sb = singles.tile([P, CC, RANK], BF16)
    s_sb = singles.tile([P, RC], F32)
    vT_sb = singles.tile([P, 2, RC, UNITS // 2], BF16)
    ctx_sb = [singles.tile([P, CC, BT], BF16, name=f"ctx{bt}") for bt in range(NBT)]
    smod = [
        [singles.tile([P, BT], F32, name=f"smod{bt}r{rc}") for rc in range(RC)]
        for bt in range(NBT)
    ]
    dt = {}
    for bt, groups in ((0, GROUPS0), (1, GROUPS1)):
        kc0 = 0
        for gi, g in enumerate(groups):
            dt[(bt, gi)] = singles.tile([P, g, BT], BF16, name=f"dt{bt}g{gi}")
            kc0 += g
    lowT = [singles.tile([P, RC, BT], BF16, name=f"lowT{bt}") for bt in range(NBT)]
    osb = [singles.tile([P, UNITS], BF16, name=f"osb{i}") for i in range(NBT * 4)]
    wu_a = singles.tile([P, P], BF16)
    wu_b = singles.tile([P, WU_N], BF16)

    # ---- DMA dispatch, emitted first so both HWDGE load rings start at ----
    # t=0. SWDGE (gpsimd) starts transfers ~3us later than HWDGE, so it
    # carries no loads — only stores (emitted in emit_out_stage), whose
    # expensive descriptor-ring drain then overlaps the PE tail. The two
    # HWDGE rings carry the loads balanced byte-for-byte in consumption
    # order: the first-needed operand of each rank group sits as early as
    # possible on one of the two rings.
    # u loads fully first on both rings (the PE needs it throughout the
    # rank stage), then the data stream flows with minimal interruption: a
    # WARM PE consumes rank data at ~0.30 GB/us while two HWDGE queues
    # deliver ~0.37 GB/us, so once warmed it never starves — but only if
    # the stream isn't broken up by other operands.
    nc.sync.dma_start(out=u_sb[:, 0:8], in_=d_u.ap()[:, 0:8])
    nc.sync.dma_start(out=dt[(0, 0)][:], in_=d_dataT.ap()[:, 0, 0:2])
    nc.sync.dma_start(out=dt[(0, 2)][:], in_=d_dataT.ap()[:, 0, 4:8])
    nc.sync.dma_start(out=dt[(0, 4)][:], in_=d_dataT.ap()[:, 0, 12:16])
    nc.sync.dma_start(out=vT_sb[:, 0], in_=d_vT.ap()[:, 0])
    nc.sync.dma_start(out=dt[(1, 0)][:], in_=d_dataT.ap()[:, 1, 0:8])
    nc.sync.dma_start(out=ctx_sb[1][:], in_=d_ctxT.ap()[:, 1])

    nc.scalar.dma_start(out=u_sb[:, 8:], in_=d_u.ap()[:, 8:])
    nc.scalar.dma_start(out=dt[(0, 1)][:], in_=d_dataT.ap()[:, 0, 2:4])
    nc.scalar.dma_start(out=dt[(0, 3)][:], in_=d_dataT.ap()[:, 0, 8:12])
    nc.scalar.dma_start(out=w_sb[:], in_=d_w.ap())
    nc.scalar.dma_start(out=s_sb[:], in_=d_s.ap())
    nc.scalar.dma_start(out=ctx_sb[0][:], in_=d_ctxT.ap()[:, 0])
    nc.scalar.dma_start(out=vT_sb[:, 1], in_=d_vT.ap()[:, 1])
    nc.scalar.dma_start(out=dt[(1, 1)][:], in_=d_dataT.ap()[:, 1, 8:16])

    # ---- HAM warm-up: dummy bf16 matmuls while the first loads stream ----
    nc.vector.memset(wu_a[:], 1.0)
    nc.vector.memset(wu_b[:], 1.0)
    wu_ps = o_psum.tile([P, MS], F32, tag="po", name="wu_ps")
    for _ in range(N_WARMUP_MM):
        nc.tensor.matmul(
            wu_ps[:, 0:WU_N], lhsT=wu_a[:], rhs=wu_b[:], start=True, stop=True
        )

    # ---- compute stages ------------------------------------------------
    pd = {}
    ps = {}

    def emit_keepers(n):
        """No-dep dummy matmuls that keep the HAM activity monitor busy
        while real matmuls are DMA-paced (a >3.4us PE idle re-throttles
        the PE clock to 1.2 GHz)."""
        for _ in range(n):
            nc.tensor.matmul(
                wu_ps[:, 0:WU_N], lhsT=wu_a[:], rhs=wu_b[:], start=True, stop=True
            )

    def emit_rank_group(bt, gi, kc0, g, first=None, last=None):
        """(u.T @ dataT) accumulation for one data chunk, both rank halves.

        `first`/`last` mark the first/last group EMITTED for this bt (the
        PSUM accumulation flags follow emission order, not kc order)."""
        first = (kc0 == 0) if first is None else first
        last = (kc0 + g == KC) if last is None else last
        if first:
            pd[bt] = [
                du_psum.tile([P, BT], F32, tag="pd", name="pd") for _ in range(RC)
            ]
        for kc in range(kc0, kc0 + g):
            for rc in range(RC):
                nc.tensor.matmul(
                    pd[bt][rc][:],
                    lhsT=u_sb[:, kc, rc * P : (rc + 1) * P],
                    rhs=dt[(bt, gi)][:, kc - kc0, :],
                    start=(first and kc == kc0),
                    stop=(last and kc == kc0 + g - 1),
                )

    def emit_smod_rc(bt, rc):
        """(w.T @ ctxT) accumulation for one rank half; the single-buf ps
        ring serializes rc1 behind rc0's evacuation automatically."""
        ps[(bt, rc)] = s_psum.tile([P, BT], F32, tag="ps", name="ps")
        for cc in range(CC):
            nc.tensor.matmul(
                ps[(bt, rc)][:],
                lhsT=w_sb[:, cc, rc * P : (rc + 1) * P],
                rhs=ctx_sb[bt][:, cc, :],
                start=(cc == 0),
                stop=(cc == CC - 1),
            )
        # smod = ps + s on the scalar engine (PSUM -> SBUF), freeing ps.
        nc.scalar.add(smod[bt][rc][:], ps[(bt, rc)][:], add=s_sb[:, rc : rc + 1])

    def emit_low(bt):
        """lowT = pd * smod on the vector engine, bf16 out."""
        for rc in range(RC):
            nc.vector.tensor_mul(
                out=lowT[bt][:, rc, :], in0=pd[bt][rc][:], in1=smod[bt][rc][:]
            )

    def emit_out_half(bt, bc, half, store=None, store_cols=False):
        """relu(lowT.T @ vT) for one 128-row chunk, one vT half (2 of the
        4 unit slices). Half-stages let the ms0/1 work run as soon as the
        first vT half lands while ms2/3 waits for the second, and spread
        the stores. `store` (on the half=1 call) emits the chunk's store
        once both halves' evacuations are in."""
        pos = [o_psum.tile([P, MS], F32, tag="po", name="po") for _ in range(2)]
        for rc in range(RC):
            for m in range(2):
                nc.tensor.matmul(
                    pos[m][:],
                    lhsT=lowT[bt][:, rc, bc * P : (bc + 1) * P],
                    rhs=vT_sb[:, half, rc, m * MS : (m + 1) * MS],
                    start=(rc == 0),
                    stop=(rc == RC - 1),
                )
        o = osb[bt * 4 + bc]
        for m in range(2):
            ms = half * 2 + m
            sl = slice(ms * MS, (ms + 1) * MS)
            if (bc + m) % 2 == 0:
                nc.scalar.activation(o[:, sl], pos[m][:], RELU)
            else:
                nc.vector.tensor_relu(out=o[:, sl], in_=pos[m][:])
        if store is None:
            return
        r0 = bt * BT + bc * P
        rows = slice(r0, r0 + P)
        if store_cols:
            # Store only this half's 1024 columns, immediately after its
            # own evacuations — the final store shrinks to the last half.
            c = slice(half * (UNITS // 2), (half + 1) * (UNITS // 2))
        else:
            c = slice(0, UNITS)
        if store == "split":
            m = (c.start + c.stop) // 2
            nc.sync.dma_start(out=d_out.ap()[rows, c.start : m], in_=o[:, c.start : m])
            nc.scalar.dma_start(out=d_out.ap()[rows, m : c.stop], in_=o[:, m : c.stop])
        elif store == "sync":
            nc.sync.dma_start(out=d_out.ap()[rows, c], in_=o[:, c])
        elif store == "scalar":
            nc.scalar.dma_start(out=d_out.ap()[rows, c], in_=o[:, c])
        else:
            nc.gpsimd.dma_start(out=d_out.ap()[rows, c], in_=o[:, c])

    # Software pipeline: PE emission ordered to match DMA arrival order;
    # bt1's rank stage fills the gaps in bt0's output stage.
    emit_rank_group(0, 0, 0, GROUPS0[0])
    emit_keepers(2)
    emit_rank_group(0, 1, 2, GROUPS0[1])
    emit_keepers(2)
    emit_rank_group(0, 2, 4, GROUPS0[2])
    emit_keepers(2)
    emit_rank_group(0, 3, 8, GROUPS0[3])
    emit_smod_rc(0, 0)
    emit_rank_group(0, 4, 12, GROUPS0[4])
    emit_smod_rc(0, 1)
    emit_low(0)
    # bt0's half-stages run as each vT half lands; rank/smod bt1 fill the
    # remaining DMA windows; bt1's half-stages close it out with stores
    # spread throughout.
    emit_out_half(0, 0, 0)
    emit_out_half(0, 1, 0)
    emit_out_half(0, 2, 0)
    emit_out_half(0, 3, 0)
    emit_out_half(0, 0, 1, store="gpsimd")
    emit_out_half(0, 1, 1, store="gpsimd")
    emit_out_half(0, 2, 1, store="gpsimd")
    emit_rank_group(1, 0, 0, GROUPS1[0])
    emit_smod_rc(1, 0)
    emit_out_half(0, 3, 1, store="gpsimd")
    emit_rank_group(1, 1, 8, GROUPS1[1])
    emit_smod_rc(1, 1)
    emit_low(1)
    emit_out_half(1, 0, 0, store="gpsimd", store_cols=True)
    emit_out_half(1, 1, 0, store="sync", store_cols=True)
    emit_out_half(1, 2, 0, store="scalar", store_cols=True)
    emit_out_half(1, 3, 0, store="sync", store_cols=True)
    emit_out_half(1, 0, 1, store="gpsimd", store_cols=True)
    emit_out_half(1, 1, 1, store="scalar", store_cols=True)
    emit_out_half(1, 2, 1, store="sync", store_cols=True)
    emit_out_half(1, 3, 1, store="split", store_cols=True)


_CACHE = {}


def build():
    if "nc" in _CACHE:
        return _CACHE["nc"]
    nc = bacc.Bacc("TRN2", target_bir_lowering=False, debug=False)
    with tile.TileContext(nc) as tc, ExitStack() as ctx:
        _emit(nc, tc, ctx)
    nc.compile()
    _CACHE["nc"] = nc
    return nc


def make_in_maps(data, context, u, s, v, w, bias):
    data16 = np.asarray(data, dtype=np.float32).astype(BF16_NP)
    ctx16 = np.asarray(context, dtype=np.float32).astype(BF16_NP)
    u16 = np.ascontiguousarray(
        np.asarray(u, dtype=np.float32).astype(BF16_NP).reshape(KC, P, RANK)
        .transpose(1, 0, 2)
    )
    w16 = np.ascontiguousarray(
        np.asarray(w, dtype=np.float32).astype(BF16_NP).reshape(CC, P, RANK)
        .transpose(1, 0, 2)
    )
    vT16 = np.ascontiguousarray(
        np.asarray(v, dtype=np.float32).astype(BF16_NP).T
        .reshape(RC, P, 2, UNITS // 2).transpose(1, 2, 0, 3)
    )
    s32 = np.ascontiguousarray(np.asarray(s, dtype=np.float32).reshape(RC, P).T)
    in_maps = []
    for c in range(NCORES):
        sl = slice(c * NB, (c + 1) * NB)
        in_maps.append(
            {
                "dataT": np.ascontiguousarray(
                    data16[sl].reshape(NBT, BT, KC, P).transpose(3, 0, 2, 1)
                ),
                "ctxT": np.ascontiguousarray(
                    ctx16[sl].reshape(NBT, BT, CC, P).transpose(3, 0, 2, 1)
                ),
                "u": u16,
                "s": s32,
                "vT": vT16,
                "w": w16,
            }
        )
    return in_maps


def kernel(data, context, u, s, v, w, bias):
    bias = np.asarray(bias, dtype=np.float32)
    if np.any(bias):
        # Reference path (bias is all-zeros per the problem spec; keep the
        # general case exact rather than specializing the device kernel).
        data = np.asarray(data, dtype=np.float32)
        context = np.asarray(context, dtype=np.float32)
        u = np.asarray(u, dtype=np.float32)
        s = np.asarray(s, dtype=np.float32)
        v = np.asarray(v, dtype=np.float32)
        w = np.asarray(w, dtype=np.float32)
        s_mod = s + context @ w
        low = (data @ u) * s_mod
        out = low @ v.T + 2.0 * bias
        return np.maximum(out, 0.0).astype(np.float32)
    nc = build()
    in_maps = make_in_maps(data, context, u, s, v, w, bias)
    res = run_bass_kernel_spmd(nc, in_maps, core_ids=list(range(NCORES)))
    return np.concatenate(
        [np.asarray(r["out"], dtype=np.float32) for r in res.results], axis=0
    )
